# revision 8
# baseline (speedup 1.0000x reference)
"""Bass/Trainium2 kernel for nn_Decoder (attention LSTM decoder, LAS-style).

Sharding: data-parallel over batch N across 8 NeuronCores (4 items/core).
Per core: sequential 200-step scan with attention + 2 LSTM cells; the
embedding gate-contribution (phase A) and the vocab projection (phase C)
are hoisted out of the loop as large efficient matmuls.

Self-contained: hardcodes the problem shapes (N=32, T=1000, L=200, V=4096,
H=KS=512, VS=256).
"""

import functools
import os
import sys

import numpy as np

for _p in ("/opt/trn_rl_repo", "/root/.axon_site/_ro/trn_rl_repo"):
    if os.path.isdir(_p) and _p not in sys.path:
        sys.path.insert(0, _p)

import ml_dtypes

import concourse.bass as bass
import concourse.tile as tile
from concourse import mybir
from concourse.bass_utils import run_bass_kernel_spmd

F32 = mybir.dt.float32
BF16 = mybir.dt.bfloat16
NP_BF16 = ml_dtypes.bfloat16

# ---------------------------------------------------------------------------
# This container's walrus build rejects instructions carrying more than two
# semaphore sync commands ("Too many sync wait commands"), which even Tile's
# own end-of-kernel drain violates. Post-process the BIR: move excess waits
# onto same-engine NoOps inserted right before the offending instruction.
import json as _json

import concourse.bass2jax as _bass2jax
import concourse.bass_utils as _bass_utils

_MAX_WAITS = 1


def _split_sync_overflow(bir_json):
    j = _json.loads(bir_json)
    ctr = 0
    for f in j["functions"]:
        for bb in f["blocks"]:
            new_insts = []
            for inst in bb["instructions"]:
                si = inst.get("sync_info")
                if si:
                    waits = si.get("on_wait") or []
                    if len(waits) > _MAX_WAITS:
                        moved = waits[: len(waits) - _MAX_WAITS]
                        si["on_wait"] = waits[len(waits) - _MAX_WAITS :]
                        for w in moved:
                            ctr += 1
                            new_insts.append({
                                "name": f"I-syncsplit-{ctr}",
                                "opcode": "NoOp",
                                "engine": inst["engine"],
                                "debug": inst.get("debug"),
                                "ins": [],
                                "outs": [],
                                "sync_info": {"on_wait": [w], "on_update": []},
                            })
                new_insts.append(inst)
            bb["instructions"] = new_insts
    return _json.dumps(j).encode()


_orig_compile_bir_kernel = _bass_utils.compile_bir_kernel


def _patched_compile_bir_kernel(bir_json, tmpdir, neff_name="file.neff"):
    return _orig_compile_bir_kernel(_split_sync_overflow(bir_json), tmpdir, neff_name)


_bass_utils.compile_bir_kernel = _patched_compile_bir_kernel
_bass2jax.compile_bir_kernel = _patched_compile_bir_kernel
# ---------------------------------------------------------------------------

N_CORES = 8
H = 512          # hidden dim (== key size KS)
KS = 512
VS = 256
G = 2048         # 4*H gate dim
V = 4096         # vocab
PART = 128

KT = KS // PART   # 4  k-tiles for energy contraction
HT = H // PART    # 4  k-tiles for h
CT = VS // PART   # 2  k-tiles for context
MT = G // PART    # 16 m-tiles for gates
ET = H // PART    # 4  k-tiles for embedding contraction
FT = (H + VS) // PART  # 6 k-tiles for output projection
VMT = V // PART   # 32 m-tiles for vocab

MASK_NEG = -100.0


def _build(B, L, TC, SC):
    """Build the per-core Bass program.

    B: items per core; L: decoder steps; TC: max t-chunks (padded T/128);
    SC: tuple of per-slot t-chunk counts (uniform across cores -> SPMD ok).
    """
    TPAD = TC * PART
    LB = L * B
    nc = bass.Bass()

    # ---- I/O ----
    key_e = nc.declare_dram_parameter("keyT", [PART, B, KT, TPAD], BF16, isOutput=False)
    val_e = nc.declare_dram_parameter("val", [PART, B, TC, CT, PART], BF16, isOutput=False)
    mask_e = nc.declare_dram_parameter("mask", [PART, B, TC], F32, isOutput=False)
    embT_e = nc.declare_dram_parameter("embT", [PART, ET, LB], BF16, isOutput=False)
    we_e = nc.declare_dram_parameter("weT", [PART, ET, MT, PART], BF16, isOutput=False)
    wc_e = nc.declare_dram_parameter("wcT", [PART, CT, MT, PART], BF16, isOutput=False)
    wh1_e = nc.declare_dram_parameter("wh1T", [PART, HT, MT, PART], BF16, isOutput=False)
    w2_e = nc.declare_dram_parameter("w2T", [PART, 2 * HT, MT, PART], BF16, isOutput=False)
    wout_e = nc.declare_dram_parameter("woutT", [VMT, PART, FT, PART], BF16, isOutput=False)
    b1_e = nc.declare_dram_parameter("b1", [PART, MT], F32, isOutput=False)
    b2_e = nc.declare_dram_parameter("b2", [PART, MT, B], F32, isOutput=False)
    bout_e = nc.declare_dram_parameter("bout", [PART, VMT], F32, isOutput=False)
    out_e = nc.declare_dram_parameter("out", [B, L, V], F32, isOutput=True)

    with tile.TileContext(nc) as tc:
        with tc.tile_pool(name="persist", bufs=1) as persist:
            # persistent SBUF state
            key_sb = persist.tile([PART, B, KT, TPAD], BF16)
            val_sb = persist.tile([PART, B, TC, CT, PART], BF16)
            mask_sb = persist.tile([PART, B, TC], F32)
            wc_sb = persist.tile([PART, CT, MT, PART], BF16)
            wh1_sb = persist.tile([PART, HT, MT, PART], BF16)
            w2_sb = persist.tile([PART, 2 * HT, MT, PART], BF16)
            b2_sb = persist.tile([PART, MT, B], F32)
            bout_sb = persist.tile([PART, VMT], F32)
            ex_sb = persist.tile([PART, L, MT, B], BF16)      # device computed
            h2h_sb = persist.tile([PART, L + 1, HT, B], BF16)  # h2 history
            ctxh_sb = persist.tile([PART, L, CT, B], BF16)     # context history
            c1_sb = persist.tile([PART, HT, B], F32)
            c2_sb = persist.tile([PART, HT, B], F32)
            h1_sb = persist.tile([PART, HT, B], BF16)
            attn_sb = persist.tile([PART, B, TC], BF16)
            acc_sb = persist.tile([PART, B], F32)
            zinv_sb = persist.tile([PART, B], F32)
            ones_sb = persist.tile([PART, PART], F32)

            # input DMAs, spread over engine queues
            nc.sync.dma_start(out=key_sb, in_=key_e[:])
            nc.gpsimd.dma_start(out=val_sb, in_=val_e[:])
            nc.gpsimd.dma_start(out=mask_sb, in_=mask_e[:])
            nc.gpsimd.dma_start(out=wc_sb, in_=wc_e[:])
            nc.scalar.dma_start(out=wh1_sb, in_=wh1_e[:])
            nc.scalar.dma_start(out=w2_sb, in_=w2_e[:])
            nc.gpsimd.dma_start(out=b2_sb, in_=b2_e[:])
            nc.gpsimd.dma_start(out=bout_sb, in_=bout_e[:])

            nc.vector.memset(ones_sb, 1.0)
            nc.vector.memset(c1_sb, 0.0)
            nc.vector.memset(c2_sb, 0.0)
            nc.vector.memset(h1_sb, 0.0)
            nc.vector.memset(h2h_sb[:, 0, :, :], 0.0)

            # ---- Phase A: ex[l] = W_e^T-stationary @ embT + b1 ----
            with (
                tc.tile_pool(name="pa_sbuf", bufs=1) as pa,
                tc.tile_pool(name="pa_psum", bufs=2, space="PSUM") as pap,
            ):
                embT_sb = pa.tile([PART, ET, LB], BF16)
                we_sb = pa.tile([PART, ET, MT, PART], BF16)
                b1_sb = pa.tile([PART, MT], F32)
                nc.gpsimd.dma_start(out=embT_sb, in_=embT_e[:])
                nc.gpsimd.dma_start(out=we_sb, in_=we_e[:])
                nc.gpsimd.dma_start(out=b1_sb, in_=b1_e[:])

                NL0 = min(128, L)  # l-chunk so B*NL0 <= 512
                for mt in range(MT):
                    for l0 in range(0, L, NL0):
                        nl = min(NL0, L - l0)
                        ps = pap.tile([PART, NL0 * B], F32, tag="pa_ps")
                        for kt in range(ET):
                            nc.tensor.matmul(
                                out=ps[:, : nl * B],
                                lhsT=we_sb[:, kt, mt, :],
                                rhs=embT_sb[:, kt, l0 * B : (l0 + nl) * B],
                                start=(kt == 0),
                                stop=(kt == ET - 1),
                            )
                        nc.scalar.activation(
                            out=ex_sb[:, l0 : l0 + nl, mt, :],
                            in_=ps[:, : nl * B].rearrange("p (l b) -> p l b", b=B),
                            func=mybir.ActivationFunctionType.Identity,
                            bias=b1_sb[:, mt : mt + 1],
                            scale=1.0,
                        )

            # ---- Sequential scan ----
            with (
                tc.tile_pool(name="lp_psum", bufs=1, space="PSUM") as lpp,
                tc.tile_pool(name="lp_psum2", bufs=1, space="PSUM") as lpp2,
                tc.tile_pool(name="lp_work", bufs=2) as lw,
            ):
                for t in range(L):
                    # energy = keyT.T @ h2  -> [t-dims(part), (b, tc)]
                    e_ps = lpp.tile([PART, B, TC], F32, tag="e_ps")
                    if t < 2:
                        nc.vector.memset(e_ps, 0.0)
                    for b in range(B):
                        for tcc in range(SC[b]):
                            for kt in range(KT):
                                nc.tensor.matmul(
                                    out=e_ps[:, b, tcc : tcc + 1],
                                    lhsT=key_sb[:, b, kt, tcc * PART : (tcc + 1) * PART],
                                    rhs=h2h_sb[:, t, kt, b : b + 1],
                                    start=(kt == 0),
                                    stop=(kt == KT - 1),
                                    skip_group_check=True,
                                )
                    # gates1 h1-part (independent of attention; keeps PE busy)
                    g1a_ps = lpp2.tile([PART, MT, B], F32, tag="g1a")
                    for mt in range(MT):
                        for kt in range(HT):
                            nc.tensor.matmul(
                                out=g1a_ps[:, mt, :],
                                lhsT=wh1_sb[:, kt, mt, :],
                                rhs=h1_sb[:, kt, :],
                                start=(kt == 0),
                                stop=(kt == HT - 1),
                                skip_group_check=True,
                            )

                    # softmax (no max-subtraction; energies are small)
                    nc.vector.tensor_add(
                        out=e_ps[:, :, :], in0=e_ps[:, :, :], in1=mask_sb[:, :, :]
                    )
                    for b in range(B):
                        nc.scalar.activation(
                            out=attn_sb[:, b, 0 : SC[b]],
                            in_=e_ps[:, b, 0 : SC[b]],
                            func=mybir.ActivationFunctionType.Exp,
                            accum_out=acc_sb[:, b : b + 1],
                        )
                    # Z (replicated across partitions) via ones-matmul
                    z_ps = lpp.tile([PART, B], F32, tag="z_ps")
                    nc.tensor.matmul(
                        out=z_ps, lhsT=ones_sb, rhs=acc_sb[:, :], start=True, stop=True,
                        skip_group_check=True,
                    )
                    nc.vector.reciprocal(out=zinv_sb, in_=z_ps)

                    # context (unnormalized) = value.T-stationary @ attn
                    c_ps = lpp.tile([PART, B, CT], F32, tag="c_ps")
                    for b in range(B):
                        for vt in range(CT):
                            for tcc in range(SC[b]):
                                nc.tensor.matmul(
                                    out=c_ps[:, b, vt : vt + 1],
                                    lhsT=val_sb[:, b, tcc, vt, :],
                                    rhs=attn_sb[:, b, tcc : tcc + 1],
                                    start=(tcc == 0),
                                    stop=(tcc == SC[b] - 1),
                                    skip_group_check=True,
                                )
                    # normalize + store context history (bf16)
                    for b in range(B):
                        nc.vector.tensor_scalar_mul(
                            out=ctxh_sb[:, t, :, b],
                            in0=c_ps[:, b, :],
                            scalar1=zinv_sb[:, b : b + 1],
                        )
                    # gates1 ctx-part (separate PSUM bank; summed on DVE later)
                    g1b_ps = lpp2.tile([PART, MT, B], F32, tag="g1b")
                    for mt in range(MT):
                        for kt in range(CT):
                            nc.tensor.matmul(
                                out=g1b_ps[:, mt, :],
                                lhsT=wc_sb[:, kt, mt, :],
                                rhs=ctxh_sb[:, t, kt, :],
                                start=(kt == 0),
                                stop=(kt == CT - 1),
                                skip_group_check=True,
                            )
                    # gates1 = g1a + ex_t + g1b
                    gsum = lw.tile([PART, MT, B], F32, tag="gsum")
                    nc.vector.tensor_add(out=gsum, in0=g1a_ps, in1=ex_sb[:, t, :, :])
                    nc.vector.tensor_add(out=gsum, in0=g1b_ps, in1=gsum)
                    sif = lw.tile([PART, 2 * HT, B], F32, tag="sif")
                    tg = lw.tile([PART, HT, B], F32, tag="tg")
                    so = lw.tile([PART, HT, B], F32, tag="so")
                    nc.scalar.activation(out=sif, in_=gsum[:, 0 : 2 * HT, :],
                                         func=mybir.ActivationFunctionType.Sigmoid)
                    nc.scalar.activation(out=tg, in_=gsum[:, 2 * HT : 3 * HT, :],
                                         func=mybir.ActivationFunctionType.Tanh)
                    nc.scalar.activation(out=so, in_=gsum[:, 3 * HT : 4 * HT, :],
                                         func=mybir.ActivationFunctionType.Sigmoid)
                    # cell 1: c1 = f*c1 + i*g ; h1 = o*tanh(c1)
                    tmp = lw.tile([PART, HT, B], F32, tag="tmp")
                    tc1 = lw.tile([PART, HT, B], F32, tag="tc1")
                    nc.vector.tensor_mul(tmp, sif[:, 0:HT, :], tg)
                    nc.vector.tensor_mul(c1_sb, sif[:, HT : 2 * HT, :], c1_sb)
                    nc.vector.tensor_add(c1_sb, c1_sb, tmp)
                    nc.scalar.activation(out=tc1, in_=c1_sb,
                                         func=mybir.ActivationFunctionType.Tanh)
                    nc.vector.tensor_mul(h1_sb, so, tc1)

                    # gates2 = W2^T-stationary @ [h1; h2]
                    g2_ps = lpp2.tile([PART, MT, B], F32, tag="g2")
                    for mt in range(MT):
                        for kt in range(2 * HT):
                            rhs = (
                                h1_sb[:, kt, :]
                                if kt < HT
                                else h2h_sb[:, t, kt - HT, :]
                            )
                            nc.tensor.matmul(
                                out=g2_ps[:, mt, :],
                                lhsT=w2_sb[:, kt, mt, :],
                                rhs=rhs,
                                start=(kt == 0),
                                stop=(kt == 2 * HT - 1),
                                skip_group_check=True,
                            )
                    gsum2 = lw.tile([PART, MT, B], F32, tag="gsum2")
                    nc.vector.tensor_add(out=gsum2, in0=g2_ps, in1=b2_sb)
                    sif2 = lw.tile([PART, 2 * HT, B], F32, tag="sif2")
                    tg2 = lw.tile([PART, HT, B], F32, tag="tg2")
                    so2 = lw.tile([PART, HT, B], F32, tag="so2")
                    nc.scalar.activation(out=sif2, in_=gsum2[:, 0 : 2 * HT, :],
                                         func=mybir.ActivationFunctionType.Sigmoid)
                    nc.scalar.activation(out=tg2, in_=gsum2[:, 2 * HT : 3 * HT, :],
                                         func=mybir.ActivationFunctionType.Tanh)
                    nc.scalar.activation(out=so2, in_=gsum2[:, 3 * HT : 4 * HT, :],
                                         func=mybir.ActivationFunctionType.Sigmoid)
                    tmp2 = lw.tile([PART, HT, B], F32, tag="tmp2")
                    tc2 = lw.tile([PART, HT, B], F32, tag="tc2")
                    nc.vector.tensor_mul(tmp2, sif2[:, 0:HT, :], tg2)
                    nc.vector.tensor_mul(c2_sb, sif2[:, HT : 2 * HT, :], c2_sb)
                    nc.vector.tensor_add(c2_sb, c2_sb, tmp2)
                    nc.scalar.activation(out=tc2, in_=c2_sb,
                                         func=mybir.ActivationFunctionType.Tanh)
                    nc.vector.tensor_mul(h2h_sb[:, t + 1, :, :], so2, tc2)

            # ---- Phase C: logits = [h2; ctx] @ w_out.T + b_out ----
            with (
                tc.tile_pool(name="pc_sbuf", bufs=4) as pc,
                tc.tile_pool(name="pc_psum", bufs=2, space="PSUM") as pcp,
            ):
                NL0 = min(128, L)
                dma_engines = [nc.sync, nc.gpsimd, nc.scalar]
                for mt in range(VMT):
                    wt = pc.tile([PART, FT, PART], BF16, tag="wt")
                    nc.sync.dma_start(out=wt, in_=wout_e[mt])
                    for ci, l0 in enumerate(range(0, L, NL0)):
                        nl = min(NL0, L - l0)
                        ps = pcp.tile([PART, NL0 * B], F32, tag="pc_ps")
                        for ft in range(FT):
                            if ft < HT:
                                rhs = h2h_sb[:, 1 + l0 : 1 + l0 + nl, ft, :]
                            else:
                                rhs = ctxh_sb[:, l0 : l0 + nl, ft - HT, :]
                            nc.tensor.matmul(
                                out=ps[:, : nl * B],
                                lhsT=wt[:, ft, :],
                                rhs=rhs,
                                start=(ft == 0),
                                stop=(ft == FT - 1),
                            )
                        st = pc.tile([PART, NL0 * B], F32, tag="st")
                        nc.scalar.activation(
                            out=st[:, : nl * B],
                            in_=ps[:, : nl * B],
                            func=mybir.ActivationFunctionType.Identity,
                            bias=bout_sb[:, mt : mt + 1],
                            scale=1.0,
                        )
                        eng = dma_engines[(mt * 2 + ci) % len(dma_engines)]
                        eng.dma_start(
                            out=out_e[:, l0 : l0 + nl, mt * PART : (mt + 1) * PART]
                            .rearrange("b l v -> v l b"),
                            in_=st[:, : nl * B].rearrange("p (l b) -> p l b", b=B),
                        )
    return nc


@functools.lru_cache(maxsize=4)
def _get_nc(B, L, TC, SC):
    return _build(B, L, TC, SC)


def _prep_and_run(key, value, emb, w_ih1, w_hh1, b_ih1, b_hh1, w_ih2, w_hh2,
                  b_ih2, b_hh2, w_out, b_out, src_lens, text, trace=False):
    N, T, _ = key.shape
    L = text.shape[1]
    B = N // N_CORES
    TPAD = ((T + PART - 1) // PART) * PART
    TC = TPAD // PART
    LB = L * B

    key = np.asarray(key, np.float32)
    value = np.asarray(value, np.float32)
    emb = np.asarray(emb, np.float32)
    lens = np.asarray(src_lens).astype(np.int64)
    text = np.asarray(text).astype(np.int64)

    # sort items by length desc, deal round-robin: slot j gets ranks [j*8, j*8+8)
    order = np.argsort(-lens, kind="stable")
    SC = tuple(
        max(1, int((lens[order[j * N_CORES]] + PART - 1) // PART)) for j in range(B)
    )
    nc = _get_nc(B, L, TC, SC)

    # shared weight tensors
    def wtile(wT, kt, mt):  # (K, M) -> (128, kt, mt, 128)
        return np.ascontiguousarray(
            wT.reshape(kt, PART, mt, PART).transpose(1, 0, 2, 3)
        ).astype(NP_BF16)

    wc_a = wtile(w_ih1[:, H:].T, CT, MT)
    wh1_a = wtile(w_hh1.T, HT, MT)
    w2_a = wtile(np.concatenate([w_ih2.T, w_hh2.T], axis=0), 2 * HT, MT)
    we_a = wtile(w_ih1[:, :H].T, ET, MT)
    wout_a = np.ascontiguousarray(
        w_out.T.reshape(FT, PART, VMT, PART).transpose(2, 1, 0, 3)
    ).astype(NP_BF16)
    b1_a = np.ascontiguousarray((b_ih1 + b_hh1).reshape(MT, PART).T).astype(np.float32)
    b2v = np.ascontiguousarray((b_ih2 + b_hh2).reshape(MT, PART).T).astype(np.float32)
    b2_a = np.ascontiguousarray(np.broadcast_to(b2v[:, :, None], (PART, MT, B))).astype(np.float32)
    bout_a = np.ascontiguousarray(b_out.reshape(VMT, PART).T).astype(np.float32)

    tgrid = np.arange(TC)[None, :] * PART + np.arange(PART)[:, None]  # (128, TC)

    in_maps = []
    for c in range(N_CORES):
        items = [int(order[j * N_CORES + c]) for j in range(B)]
        kp = np.zeros((B, TPAD, KS), np.float32)
        kp[:, :T] = key[items]
        key_a = np.ascontiguousarray(
            kp.transpose(2, 0, 1).reshape(KT, PART, B, TPAD).transpose(1, 2, 0, 3)
        ).astype(NP_BF16)
        vp = np.zeros((B, TPAD, VS), np.float32)
        vp[:, :T] = value[items]
        val_a = np.ascontiguousarray(
            vp.reshape(B, TC, PART, CT, PART).transpose(2, 0, 1, 3, 4)
        ).astype(NP_BF16)
        lens_c = lens[items]
        mask_a = np.where(
            tgrid[:, None, :] < lens_c[None, :, None], 0.0, MASK_NEG
        ).astype(np.float32)
        e = emb[text[items]]  # (B, L, H)
        embT_a = np.ascontiguousarray(
            e.transpose(2, 1, 0).reshape(ET, PART, LB).transpose(1, 0, 2)
        ).astype(NP_BF16)
        in_maps.append({
            "keyT": key_a, "val": val_a, "mask": mask_a, "embT": embT_a,
            "weT": we_a, "wcT": wc_a, "wh1T": wh1_a, "w2T": w2_a,
            "woutT": wout_a, "b1": b1_a, "b2": b2_a, "bout": bout_a,
        })

    res = run_bass_kernel_spmd(
        nc, in_maps, core_ids=list(range(N_CORES)), trace=trace
    )
    out_full = np.empty((N, L, V), np.float32)
    for c in range(N_CORES):
        oc = res.results[c]["out"]
        for j in range(B):
            out_full[order[j * N_CORES + c]] = oc[j]
    if trace:
        return out_full, res
    return out_full


def kernel(**inputs):
    return _prep_and_run(**inputs)
